# revision 1
# baseline (speedup 1.0000x reference)
"""GCN autoencoder (2x GCN layer + inner-product decoder) on 8 TRN2 NeuronCores.

Problem (full shapes):
    x [8192, 512] f32, w1 [512, 256] f32, w2 [256, 16] f32,
    edge_weight [262144] f32, row/col [262144] i32
    h1  = relu(segment_sum((x @ w1)[col] * ew, row, 8192))     # [8192, 256]
    z   = segment_sum((h1 @ w2)[col] * ew, row, 8192)          # [8192, 16]
    adj = z @ z.T                                              # [8192, 8192]

Strategy (node / segment-id sharding, 1024 destination rows per core):
  The COO graph is converted on host into a dense adjacency matrix
  A[r, c] = sum of edge_weight over edges (row=r, col=c), stored bf16;
  both GCN aggregations become dense matmuls.
    P1  support1 = x @ w1           (full, computed redundantly per core)
    P2  h1_c^T = relu(support1^T @ A^T[:, rows_c])   row-shard of A
    P4  s2_c   = h1_c @ w2                            local
    P5  zpart  = s2_c^T @ A^T[rows_c-as-sources, :]  column-shard of A:
        partial z^T over ALL destinations from this core's source block
    P6  ReduceScatter(add) over zpart -> z_c^T; AllGather -> z^T (bf16)
    P7  adj_c = z_c @ z^T  (bf16 matmuls, K zero-padded to 128), row-slice
        of the output written as fp32.
  All TensorEngine work is bf16 (final rel err ~1e-3).
"""

import os
import sys

import numpy as np

if "/opt/trn_rl_repo" not in sys.path:
    sys.path.insert(0, "/opt/trn_rl_repo")

import ml_dtypes

import concourse.bass as bass
import concourse.mybir as mybir
import concourse.tile as tile
from concourse import bacc
from concourse.bass_utils import run_bass_kernel_spmd

N = 8192          # nodes
D_IN = 512        # input features
D_H = 256         # hidden features
D_Z = 16          # latent features
NCORES = 8
R = N // NCORES   # 1024 destination rows per core
P = 128

BF = mybir.dt.bfloat16
F32 = mybir.dt.float32

# stash for test harness introspection (exec_time_ns etc.)
LAST_RESULTS = None
_NC_CACHE = None


def _build_kernel(phases=7):
    nc = bacc.Bacc("TRN2", target_bir_lowering=False, debug=False,
                   num_devices=NCORES)

    xT = nc.dram_tensor("xT", [D_IN, N], BF, kind="ExternalInput").ap()
    w1 = nc.dram_tensor("w1", [D_IN, D_H], BF, kind="ExternalInput").ap()
    w2 = nc.dram_tensor("w2", [D_H, D_Z], BF, kind="ExternalInput").ap()
    # A^T row-shard (sources x own-destinations), partition-major:
    # ATr[p, k, r] = A^T[k*128 + p, core*R + r]
    ATr = nc.dram_tensor("ATr", [P, N // P, R], BF, kind="ExternalInput").ap()
    # A^T column-shard (own-sources x all destinations), blocked for P5:
    # ATc[p, nb, kk, j] = A^T[core*R + kk*128 + p, nb*512 + j]
    ATc = nc.dram_tensor("ATc", [P, N // 512, R // P, 512], BF,
                         kind="ExternalInput").ap()
    adj = nc.dram_tensor("adj", [R, N], F32, kind="ExternalOutput").ap()

    with tile.TileContext(nc) as tc:
        _body(tc, xT, w1, w2, ATr, ATc, adj, phases)
    nc.compile()
    return nc


def _body(tc, xT, w1, w2, ATr, ATc, adj, phases=7):
    nc = tc.nc
    KX = D_IN // P          # 4 k-chunks over input features
    MCH = N // P            # 64 node chunks
    DH_CH = D_H // P        # 2 chunks over hidden features
    KCH = N // P            # 64 k-chunks over source nodes
    AGRP = 4                # ATr k-chunks fetched per DMA (1 MiB)
    NB = N // 512           # 16 column blocks of the z / output
    RB = R // P             # 8 row blocks per core
    KKC = R // P            # 8 source chunks in own block (P5)

    xT_v = xT.rearrange("(k p) n -> p k n", p=P)                  # [128, 4, 8192]
    w1_v = w1.rearrange("(k p) n -> p k n", p=P)                  # [128, 4, 256]
    w2_v = w2.rearrange("(k p) n -> p k n", p=P)                  # [128, 2, 16]

    with (
        tc.tile_pool(name="const", bufs=1) as const,
        tc.tile_pool(name="persist", bufs=1) as persist,
        tc.tile_pool(name="xstream", bufs=2) as xstream,
        tc.tile_pool(name="astream", bufs=3) as astream,
        tc.tile_pool(name="acstream", bufs=6) as acstream,
        tc.tile_pool(name="outbuf", bufs=5) as outbuf,
        tc.tile_pool(name="psum_rot", bufs=2, space="PSUM") as psum_rot,
        tc.tile_pool(name="psum_acc", bufs=1, space="PSUM") as psum_acc,
        tc.tile_pool(name="dram", bufs=1, space="DRAM") as dram,
    ):
        # ---- constants ----
        w1s = const.tile([P, KX, D_H], BF)
        nc.sync.dma_start(w1s[:], w1_v[:])
        w2s = const.tile([P, DH_CH, D_Z], BF)
        nc.sync.dma_start(w2s[:], w2_v[:])

        # ---- persistent activations ----
        support1 = persist.tile([P, MCH, D_H], BF)       # (x@w1)    [8192, 256]
        h1T = persist.tile([P, DH_CH, R], BF)            # h1_c^T    [256, 1024]
        s2o = persist.tile([P, KKC, D_Z], BF)            # s2_c      [1024, 16]
        zTp = persist.tile([D_Z, NCORES, R], BF)         # partial z^T [16, 8192]
        zT_i = persist.tile([P, R], BF)                  # z_c^T K-padded
        zT_full = persist.tile([P, NCORES, R], BF)       # z^T   K-padded
        nc.gpsimd.memset(zT_i[:], 0)
        nc.gpsimd.memset(zT_full[:], 0)

        # ================= Phase 1: support1 = x @ w1 (full, redundant) ====
        # first fetch is small so the PE starts early; the rest stream in
        # 8-chunk groups
        groups = [(0, 2), (2, 2), (4, 4)] + [(g, 8) for g in range(8, MCH, 8)]
        for (m0, gw) in groups:
            xts = xstream.tile([P, KX, 8 * P], BF, tag="xts")
            nc.sync.dma_start(xts[:, :, :gw * P],
                              xT_v[:, :, m0 * P:(m0 + gw) * P])
            for ml in range(gw):
                m = m0 + ml
                s1p = psum_rot.tile([P, D_H], F32, tag="psrot")
                for k in range(KX):
                    nc.tensor.matmul(
                        s1p[:], lhsT=xts[:, k, ml * P:(ml + 1) * P],
                        rhs=w1s[:, k], start=(k == 0), stop=(k == KX - 1))
                nc.vector.tensor_copy(support1[:, m], s1p[:])

        # ================= Phase 2: h1_c^T = relu(support1^T @ ATr) =======
        ph = [[psum_acc.tile([P, 512], F32, name=f"ph_{dh}_{nn}",
                             tag=f"ph_{dh}_{nn}")
               for nn in range(2)] for dh in range(2)]
        for g in range(KCH // AGRP):
            ats = astream.tile([P, AGRP, R], BF, tag="ats")
            nc.sync.dma_start(ats[:], ATr[:, g * AGRP:(g + 1) * AGRP, :])
            for c in range(AGRP):
                k = g * AGRP + c
                for dh in range(DH_CH):
                    for nn in range(2):
                        nc.tensor.matmul(
                            ph[dh][nn][:],
                            lhsT=support1[:, k, dh * P:(dh + 1) * P],
                            rhs=ats[:, c, nn * 512:(nn + 1) * 512],
                            start=(k == 0), stop=(k == KCH - 1))
        for dh in range(DH_CH):
            for nn in range(2):
                nc.vector.tensor_scalar_max(
                    h1T[:, dh, nn * 512:(nn + 1) * 512], ph[dh][nn][:], 0.0)

        if phases < 3:
            return
        # ================= Phase 4: s2_c = h1_c @ w2 (local) ==============
        for ml in range(KKC):
            s2p = psum_rot.tile([P, D_Z], F32, tag="psrot")
            for dh in range(DH_CH):
                nc.tensor.matmul(
                    s2p[:], lhsT=h1T[:, dh, ml * P:(ml + 1) * P],
                    rhs=w2s[:, dh], start=(dh == 0), stop=(dh == DH_CH - 1))
            nc.vector.tensor_copy(s2o[:, ml], s2p[:])

        if phases < 4:
            return
        # ================= Phase 5: zpart^T = s2_c^T @ ATc ================
        for nb in range(NB):
            acs = acstream.tile([P, KKC, 512], BF, tag="acs")
            nc.sync.dma_start(acs[:], ATc[:, nb])
            pz = psum_acc.tile([D_Z, 512], F32, name=f"pz_{nb}",
                               tag=f"ph_{nb % 2}_{(nb // 2) % 2}")
            for kk in range(KKC):
                nc.tensor.matmul(
                    pz[:], lhsT=s2o[:, kk], rhs=acs[:, kk],
                    start=(kk == 0), stop=(kk == KKC - 1))
            nc.vector.tensor_copy(
                zTp[:].rearrange("p g r -> p (g r)")[:, nb * 512:(nb + 1) * 512],
                pz[:])

        if phases < 5:
            return
        # ================= Phase 6: ReduceScatter + AllGather z^T =========
        cc_rs_in = dram.tile([NCORES, D_Z, R], BF)
        cc_rs_out = dram.tile([D_Z, R], BF)
        cc_ag_out = dram.tile([NCORES, D_Z, R], BF, addr_space="Shared")
        nc.gpsimd.dma_start(cc_rs_in[:].rearrange("g p r -> p g r"), zTp[:])
        nc.gpsimd.collective_compute(
            "ReduceScatter", mybir.AluOpType.add,
            replica_groups=[list(range(NCORES))],
            ins=[cc_rs_in[:].opt()], outs=[cc_rs_out[:].opt()])
        nc.gpsimd.dma_start(zT_i[:D_Z], cc_rs_out[:])
        if phases < 6:
            return
        nc.gpsimd.collective_compute(
            "AllGather", mybir.AluOpType.bypass,
            replica_groups=[list(range(NCORES))],
            ins=[cc_rs_out[:].opt()], outs=[cc_ag_out[:].opt()])
        nc.gpsimd.dma_start(zT_full[:D_Z], cc_ag_out[:].rearrange("g p r -> p g r"))

        if phases < 7:
            return
        # ================= Phase 7: adj_c = z_c @ z^T =====================
        zT_full_f = zT_full[:].rearrange("p g r -> p (g r)")
        OWID = 2048  # output DMA chunk width (1 MiB per transfer)
        for mb in range(RB):
            for og in range(N // OWID):
                rowbuf = outbuf.tile([P, OWID], F32, tag="rowbuf")
                for ol in range(OWID // 512):
                    nb = og * (OWID // 512) + ol
                    po = psum_acc.tile(
                        [P, 512], F32, name=f"po_{mb}_{nb}",
                        tag=f"ph_{nb % 2}_{(nb // 2) % 2}")
                    nc.tensor.matmul(
                        po[:], lhsT=zT_i[:, mb * P:(mb + 1) * P],
                        rhs=zT_full_f[:, nb * 512:(nb + 1) * 512],
                        start=True, stop=True)
                    # split PSUM drains between DVE and ACT (3:1)
                    dst = rowbuf[:, ol * 512:(ol + 1) * 512]
                    if nb % 4 == 3:
                        nc.scalar.copy(dst, po[:])
                    else:
                        nc.vector.tensor_copy(dst, po[:])
                nc.sync.dma_start(
                    adj[mb * P:(mb + 1) * P, og * OWID:(og + 1) * OWID], rowbuf[:])


def _get_nc():
    global _NC_CACHE
    phases = int(os.environ.get("BASS_KERNEL_PHASES", "7"))
    if _NC_CACHE is None or _NC_CACHE[0] != phases:
        _NC_CACHE = (phases, _build_kernel(phases))
    return _NC_CACHE[1]


def kernel(x, w1, w2, edge_weight, row, col):
    global LAST_RESULTS
    x = np.asarray(x, dtype=np.float32)
    w1 = np.asarray(w1, dtype=np.float32)
    w2 = np.asarray(w2, dtype=np.float32)
    edge_weight = np.asarray(edge_weight, dtype=np.float32)
    row = np.asarray(row, dtype=np.int64)
    col = np.asarray(col, dtype=np.int64)

    bf16 = ml_dtypes.bfloat16

    # Dense A^T: AT[c, r] = sum of edge_weight over edges with (row=r, col=c)
    # i.e. AT[source, dest]
    AT_dense = np.zeros((N, N), dtype=np.float32)
    np.add.at(AT_dense, (col, row), edge_weight)
    AT_bf = AT_dense.astype(bf16)

    xT_bf = np.ascontiguousarray(x.T).astype(bf16)
    w1_bf = w1.astype(bf16)
    w2_bf = w2.astype(bf16)

    in_maps = []
    for c in range(NCORES):
        # row shard: [src, own-dest] -> partition-major [128, 64, R]
        atr = AT_bf[:, c * R:(c + 1) * R]                 # [8192, 1024]
        atr = np.ascontiguousarray(
            atr.reshape(N // P, P, R).transpose(1, 0, 2))  # [128, 64, 1024]
        # col shard: [own-src, all-dest] -> [128, 16, 8, 512]
        atc = AT_bf[c * R:(c + 1) * R, :]                 # [1024, 8192]
        atc = np.ascontiguousarray(
            atc.reshape(R // P, P, N // 512, 512).transpose(1, 2, 0, 3))
        in_maps.append({
            "xT": xT_bf,
            "w1": w1_bf,
            "w2": w2_bf,
            "ATr": atr,
            "ATc": atc,
        })

    nc = _get_nc()
    print("kernel: launching on 8 cores", flush=True)
    res = run_bass_kernel_spmd(nc, in_maps, core_ids=list(range(NCORES)))
    print("kernel: run complete", flush=True)
    LAST_RESULTS = res
    adj = np.concatenate([res.results[c]["adj"] for c in range(NCORES)], axis=0)
    return np.ascontiguousarray(adj.astype(np.float32))



# revision 3
# speedup vs baseline: 1.1984x; 1.1984x over previous
"""GCN autoencoder (2x GCN layer + inner-product decoder) on 8 TRN2 NeuronCores.

Problem (full shapes):
    x [8192, 512] f32, w1 [512, 256] f32, w2 [256, 16] f32,
    edge_weight [262144] f32, row/col [262144] i32
    h1  = relu(segment_sum((x @ w1)[col] * ew, row, 8192))     # [8192, 256]
    z   = segment_sum((h1 @ w2)[col] * ew, row, 8192)          # [8192, 16]
    adj = z @ z.T                                              # [8192, 8192]

Strategy (node / destination-row sharding, 1024 rows per core):
  The COO graph is converted on host into a dense adjacency matrix
  A[r, c] = sum of edge_weight over edges (row=r, col=c), stored bf16;
  both GCN aggregations become dense matmuls against the SAME row shard
  A^T[:, own_dest] which is streamed once into SBUF and kept resident.
    P1  s1 chunk  = x_chunk @ w1          (redundant per core, pipelined)
    P2  h1_c^T   += s1_chunk^T @ ATr_chunk  (accumulate over 64 chunks)
    P3  s2_c      = h1_c @ w2               (local, [1024, 16])
    AG1 AllGather s2 -> s2 full [8192, 16]  (32 KiB payload)
    P5  z_c^T     = s2^T @ ATr  (SBUF-resident reuse, [16, 1024])
    AG2 AllGather z -> z^T full [16, 8192]  (32 KiB payload)
    P7  adj_c     = z_c @ z^T  (K=16 matmuls), written as bf16; the host
        converts to fp32 (adds ~0.2% RMS, well within the 2e-2 gate).
  A tiny warmup AllGather is issued at kernel start so the collective
  bootstrap cost overlaps the input streams.
"""

import os
import sys

import numpy as np

if "/opt/trn_rl_repo" not in sys.path:
    sys.path.insert(0, "/opt/trn_rl_repo")

import ml_dtypes

import concourse.bass as bass
import concourse.mybir as mybir
import concourse.tile as tile
from concourse import bacc
from concourse.bass_utils import run_bass_kernel_spmd

N = 8192          # nodes
D_IN = 512        # input features
D_H = 256         # hidden features
D_Z = 16          # latent features
NCORES = 8
R = N // NCORES   # 1024 destination rows per core
P = 128

BF = mybir.dt.bfloat16
F32 = mybir.dt.float32

# stash for test harness introspection (exec_time_ns etc.)
LAST_RESULTS = None
_NC_CACHE = None


def _build_kernel(phases=7):
    nc = bacc.Bacc("TRN2", target_bir_lowering=False, debug=False,
                   num_devices=NCORES)

    xT = nc.dram_tensor("xT", [D_IN, N], BF, kind="ExternalInput").ap()
    w1 = nc.dram_tensor("w1", [D_IN, D_H], BF, kind="ExternalInput").ap()
    w2 = nc.dram_tensor("w2", [D_H, D_Z], BF, kind="ExternalInput").ap()
    # A^T row-shard (sources x own-destinations), partition-major:
    # ATr[p, k, r] = A^T[k*128 + p, core*R + r]
    ATr = nc.dram_tensor("ATr", [P, N // P, R], BF, kind="ExternalInput").ap()
    adjb = nc.dram_tensor("adjb", [R, N], BF, kind="ExternalOutput").ap()

    with tile.TileContext(nc) as tc:
        _body(tc, xT, w1, w2, ATr, adjb, phases)
    nc.compile()
    return nc


def _body(tc, xT, w1, w2, ATr, adjb, phases=7):
    nc = tc.nc
    KX = D_IN // P          # 4 k-chunks over input features
    KCH = N // P            # 64 node chunks (sources / P1 rows)
    DH_CH = D_H // P        # 2 chunks over hidden features
    GRP = 4                 # node chunks per DMA group
    NGRP = KCH // GRP       # 16 groups
    RB = R // P             # 8 own row blocks

    xT_v = xT.rearrange("(k p) n -> p k n", p=P)                  # [128, 4, 8192]
    w1_v = w1.rearrange("(k p) n -> p k n", p=P)                  # [128, 4, 256]
    w2_v = w2.rearrange("(k p) n -> p k n", p=P)                  # [128, 2, 16]

    with (
        tc.tile_pool(name="const", bufs=1) as const,
        tc.tile_pool(name="persist", bufs=1) as persist,
        tc.tile_pool(name="xstream", bufs=3) as xstream,
        tc.tile_pool(name="s1rot", bufs=3) as s1rot,
        tc.tile_pool(name="outbuf", bufs=4) as outbuf,
        tc.tile_pool(name="psum_rot", bufs=2, space="PSUM") as psum_rot,
        tc.tile_pool(name="psum_acc", bufs=1, space="PSUM") as psum_acc,
        tc.tile_pool(name="dram", bufs=1, space="DRAM") as dram,
    ):
        # ---- warmup collective: absorbs CC bootstrap off the critical path
        warm_in = dram.tile([1, D_Z], BF)
        warm_out = dram.tile([NCORES, D_Z], BF, addr_space="Shared")
        nc.gpsimd.collective_compute(
            "AllGather", mybir.AluOpType.bypass,
            replica_groups=[list(range(NCORES))],
            ins=[warm_in[:].opt()], outs=[warm_out[:].opt()])

        # ---- constants ----
        w1s = const.tile([P, KX, D_H], BF)
        nc.sync.dma_start(w1s[:], w1_v[:])
        w2s = const.tile([P, DH_CH, D_Z], BF)
        nc.sync.dma_start(w2s[:], w2_v[:])

        # ---- persistent tiles ----
        atr_sb = persist.tile([P, KCH, R], BF)           # A^T shard, 128 KiB/part
        h1T = persist.tile([P, DH_CH, R], BF)            # h1_c^T    [256, 1024]
        s2o = persist.tile([P, RB, D_Z], BF)             # s2_c      [1024, 16]
        s2f = persist.tile([P, KCH, D_Z], BF)            # s2 full   [8192, 16]
        zT_c = persist.tile([D_Z, R], BF)                # z_c^T     [16, 1024]
        zT_sb = persist.tile([D_Z, NCORES, R], BF)       # z^T full  [16, 8192]

        # ========== Phase 1+2 pipelined: s1 = x@w1 ; h1_c^T += s1^T @ ATr ==
        ph = [[psum_acc.tile([P, 512], F32, name=f"ph_{dh}_{nn}",
                             tag=f"ph_{dh}_{nn}")
               for nn in range(2)] for dh in range(2)]
        for g in range(NGRP):
            m0 = g * GRP
            xts = xstream.tile([P, KX, GRP * P], BF, tag="xts")
            nc.sync.dma_start(xts[:], xT_v[:, :, m0 * P:(m0 + GRP) * P])
            nc.sync.dma_start(atr_sb[:, m0:m0 + GRP, :],
                              ATr[:, m0:m0 + GRP, :])
            for ml in range(GRP):
                m = m0 + ml
                # P1: s1 chunk m = x_m @ w1  -> [128 nodes, 256]
                s1p = psum_rot.tile([P, D_H], F32, tag="psrot")
                for k in range(KX):
                    nc.tensor.matmul(
                        s1p[:], lhsT=xts[:, k, ml * P:(ml + 1) * P],
                        rhs=w1s[:, k], start=(k == 0), stop=(k == KX - 1))
                s1c = s1rot.tile([P, D_H], BF, tag="s1c")
                nc.vector.tensor_copy(s1c[:], s1p[:])
                # P2: accumulate h1_c^T over source chunk m
                for dh in range(DH_CH):
                    for nn in range(2):
                        nc.tensor.matmul(
                            ph[dh][nn][:],
                            lhsT=s1c[:, dh * P:(dh + 1) * P],
                            rhs=atr_sb[:, m, nn * 512:(nn + 1) * 512],
                            start=(m == 0), stop=(m == KCH - 1))
        for dh in range(DH_CH):
            for nn in range(2):
                nc.vector.tensor_scalar_max(
                    h1T[:, dh, nn * 512:(nn + 1) * 512], ph[dh][nn][:], 0.0)

        if phases < 3:
            return
        # ========== Phase 3: s2_c = h1_c @ w2 (local) =====================
        for ml in range(RB):
            s2p = psum_rot.tile([P, D_Z], F32, tag="psrot")
            for dh in range(DH_CH):
                nc.tensor.matmul(
                    s2p[:], lhsT=h1T[:, dh, ml * P:(ml + 1) * P],
                    rhs=w2s[:, dh], start=(dh == 0), stop=(dh == DH_CH - 1))
            nc.vector.tensor_copy(s2o[:, ml], s2p[:])

        if phases < 4:
            return
        # ========== AG1: AllGather s2 -> s2 full ==========================
        ag1_in = dram.tile([R, D_Z], BF)
        ag1_out = dram.tile([NCORES, R, D_Z], BF, addr_space="Shared")
        nc.gpsimd.dma_start(ag1_in[:].rearrange("(ml p) j -> p ml j", p=P),
                            s2o[:])
        nc.gpsimd.collective_compute(
            "AllGather", mybir.AluOpType.bypass,
            replica_groups=[list(range(NCORES))],
            ins=[ag1_in[:].opt()], outs=[ag1_out[:].opt()])
        nc.gpsimd.dma_start(
            s2f[:],
            ag1_out[:].rearrange("c (kk p) j -> p (c kk) j", p=P))

        if phases < 5:
            return
        # ========== Phase 5: z_c^T = s2^T @ ATr (SBUF-resident reuse) =====
        pz = [psum_acc.tile([D_Z, 512], F32, name=f"pz_{nn}",
                            tag=f"ph_0_{nn}") for nn in range(2)]
        for k in range(KCH):
            for nn in range(2):
                nc.tensor.matmul(
                    pz[nn][:], lhsT=s2f[:, k],
                    rhs=atr_sb[:, k, nn * 512:(nn + 1) * 512],
                    start=(k == 0), stop=(k == KCH - 1))
        for nn in range(2):
            nc.vector.tensor_copy(zT_c[:, nn * 512:(nn + 1) * 512], pz[nn][:])

        if phases < 6:
            return
        # ========== AG2: AllGather z -> z^T full ==========================
        ag2_in = dram.tile([D_Z, R], BF)
        ag2_out = dram.tile([NCORES, D_Z, R], BF, addr_space="Shared")
        nc.gpsimd.dma_start(ag2_in[:], zT_c[:])
        nc.gpsimd.collective_compute(
            "AllGather", mybir.AluOpType.bypass,
            replica_groups=[list(range(NCORES))],
            ins=[ag2_in[:].opt()], outs=[ag2_out[:].opt()])
        nc.gpsimd.dma_start(zT_sb[:], ag2_out[:].rearrange("c i r -> i c r"))

        if phases < 7:
            return
        # ========== Phase 7: adj_c = z_c @ z^T (bf16 out) =================
        zT_flat = zT_sb[:].rearrange("i c r -> i (c r)")
        OWID = 2048  # output DMA chunk width (0.5 MiB per transfer)
        drain_cycle = [nc.vector.tensor_copy, nc.scalar.copy,
                       nc.vector.tensor_copy, nc.scalar.copy]
        for mb in range(RB):
            for og in range(N // OWID):
                rowbuf = outbuf.tile([P, OWID], BF, tag="rowbuf")
                for ol in range(OWID // 512):
                    nb = og * (OWID // 512) + ol
                    po = psum_acc.tile(
                        [P, 512], F32, name=f"po_{mb}_{nb}",
                        tag=f"ph_{nb % 2}_{(nb // 2) % 2}")
                    nc.tensor.matmul(
                        po[:], lhsT=zT_c[:, mb * P:(mb + 1) * P],
                        rhs=zT_flat[:, nb * 512:(nb + 1) * 512],
                        start=True, stop=True)
                    # split PSUM drains across DVE/ACT/Pool
                    drain_cycle[ol](rowbuf[:, ol * 512:(ol + 1) * 512], po[:])
                nc.sync.dma_start(
                    adjb[mb * P:(mb + 1) * P, og * OWID:(og + 1) * OWID],
                    rowbuf[:])


def _get_nc():
    global _NC_CACHE
    phases = int(os.environ.get("BASS_KERNEL_PHASES", "7"))
    if _NC_CACHE is None or _NC_CACHE[0] != phases:
        _NC_CACHE = (phases, _build_kernel(phases))
    return _NC_CACHE[1]


def kernel(x, w1, w2, edge_weight, row, col):
    global LAST_RESULTS
    x = np.asarray(x, dtype=np.float32)
    w1 = np.asarray(w1, dtype=np.float32)
    w2 = np.asarray(w2, dtype=np.float32)
    edge_weight = np.asarray(edge_weight, dtype=np.float32)
    row = np.asarray(row, dtype=np.int64)
    col = np.asarray(col, dtype=np.int64)

    bf16 = ml_dtypes.bfloat16

    # Dense A^T: AT[c, r] = sum of edge_weight over edges with (row=r, col=c)
    # i.e. AT[source, dest]
    AT_dense = np.zeros((N, N), dtype=np.float32)
    np.add.at(AT_dense, (col, row), edge_weight)
    AT_bf = AT_dense.astype(bf16)

    xT_bf = np.ascontiguousarray(x.T).astype(bf16)
    w1_bf = w1.astype(bf16)
    w2_bf = w2.astype(bf16)

    in_maps = []
    for c in range(NCORES):
        # row shard: [src, own-dest] -> partition-major [128, 64, R]
        atr = AT_bf[:, c * R:(c + 1) * R]                 # [8192, 1024]
        atr = np.ascontiguousarray(
            atr.reshape(N // P, P, R).transpose(1, 0, 2))  # [128, 64, 1024]
        in_maps.append({
            "xT": xT_bf,
            "w1": w1_bf,
            "w2": w2_bf,
            "ATr": atr,
        })

    nc = _get_nc()
    print("kernel: launching on 8 cores", flush=True)
    res = run_bass_kernel_spmd(nc, in_maps, core_ids=list(range(NCORES)))
    print("kernel: run complete", flush=True)
    LAST_RESULTS = res
    adj = np.concatenate([res.results[c]["adjb"] for c in range(NCORES)],
                         axis=0)
    return np.ascontiguousarray(adj.astype(np.float32))
